# revision 25
# baseline (speedup 1.0000x reference)
"""BiMamba block Trainium2 kernel.

Sharding: 8 cores = 2 branches (fwd/bwd) x 2 batches x 2 d_inner-halves.
Each core runs an identical SPMD Bass/Tile program on its shard:
  LayerNorm -> in_proj -> causal depthwise conv -> SiLU -> x_proj ->
  dt/B/C -> selective scan (tensor_tensor_scan) -> gating -> out_proj partial.
Host slices/permutes inputs per core and sums the partial outputs.

d_inner is permuted on the host so that each core's own half occupies
channel chunks 0..7 (program structure is identical across cores).
"""
import sys

sys.path.insert(0, "/opt/trn_rl_repo")

import contextlib

import numpy as np

import concourse.bacc as bacc
import concourse.bass as bass
import concourse.tile as tile
from concourse import mybir

F32 = mybir.dt.float32
F32R = mybir.dt.float32r
BF16 = mybir.dt.bfloat16
FP16 = mybir.dt.float16
ALU = mybir.AluOpType
ACTF = mybir.ActivationFunctionType

D_MODEL = 1024
D_STATE = 16
D_CONV = 4
D_INNER = 2048
DT_RANK = 64
BATCH, SEQ = 2, 1024
HALF = D_INNER // 2          # 1024 channels per core
L = SEQ                      # 1024 timesteps
NJ = D_INNER // 128          # 16 u chunks
NJH = HALF // 128            # 8 chunks in our half
NK = D_MODEL // 128          # 8 k-tiles for d_model contraction
N_CORES = 8
TC = 512                     # psum free chunk

# ---------------------------------------------------------------------------
# device program
# ---------------------------------------------------------------------------


def build_program(silu_compat=False):
    # silu_compat: CoreSim lacks the Silu LUT; emulate via Sigmoid + mult.
    nc = bacc.Bacc("TRN2", target_bir_lowering=False, debug=True)

    # --- DRAM I/O (per core shapes) ---
    xT_d = nc.dram_tensor("xT", [D_MODEL, L], F32, kind="ExternalInput")
    gamma_d = nc.dram_tensor("gamma_col", [D_MODEL, 1], F32, kind="ExternalInput")
    beta_d = nc.dram_tensor("beta_col", [D_MODEL, 1], F32, kind="ExternalInput")
    wuT_d = nc.dram_tensor("wuT", [D_MODEL, D_INNER], F32, kind="ExternalInput")
    wzT_d = nc.dram_tensor("wzT", [D_MODEL, HALF], F32, kind="ExternalInput")
    convdiag_d = nc.dram_tensor("convdiag", [NJ, D_CONV, 128, 128], F32, kind="ExternalInput")
    convb_d = nc.dram_tensor("convb_col", [D_INNER, 1], F32, kind="ExternalInput")
    xprojT_d = nc.dram_tensor("xprojT", [D_INNER, 96], F32, kind="ExternalInput")
    dtwT_d = nc.dram_tensor("dtwT", [DT_RANK, HALF], F32, kind="ExternalInput")
    dtb_d = nc.dram_tensor("dtb_col", [HALF, 1], F32, kind="ExternalInput")
    A_d = nc.dram_tensor("A_half", [HALF, D_STATE], F32, kind="ExternalInput")
    D_col_d = nc.dram_tensor("D_col", [HALF, 1], F32, kind="ExternalInput")
    owT_d = nc.dram_tensor("owT", [HALF, D_MODEL], F32, kind="ExternalInput")
    ones_d = nc.dram_tensor("ones_col", [128, 1], F32, kind="ExternalInput")
    ident_d = nc.dram_tensor("ident_bf", [128, 128], FP16, kind="ExternalInput")
    outp_d = nc.dram_tensor("outp", [L, D_MODEL], F32, kind="ExternalOutput")

    with tile.TileContext(nc) as tc:
        with contextlib.ExitStack() as stack:
            consts = stack.enter_context(tc.tile_pool(name="consts", bufs=1))
            persist = stack.enter_context(tc.tile_pool(name="persist", bufs=1))
            psA = stack.enter_context(tc.tile_pool(name="psA", bufs=4, space="PSUM"))
            dramp = stack.enter_context(tc.tile_pool(name="dramp", bufs=1, space="DRAM"))

            # ---- constants / small resident tensors ----
            gamma = consts.tile([128, NK, 1], F32, tag="gamma")
            nc.sync.dma_start(out=gamma, in_=gamma_d[:].rearrange("(k p) o -> p k o", p=128))
            beta = consts.tile([128, NK, 1], F32, tag="beta")
            nc.sync.dma_start(out=beta, in_=beta_d[:].rearrange("(k p) o -> p k o", p=128))
            convb = consts.tile([128, NJ, 1], F32, tag="convb")
            nc.sync.dma_start(out=convb, in_=convb_d[:].rearrange("(j p) o -> p j o", p=128))
            dtb = consts.tile([128, NJH, 1], F32, tag="dtb")
            nc.sync.dma_start(out=dtb, in_=dtb_d[:].rearrange("(j p) o -> p j o", p=128))
            A_sb = consts.tile([128, NJH, D_STATE], F32, tag="A")
            nc.sync.dma_start(out=A_sb, in_=A_d[:].rearrange("(j p) n -> p j n", p=128))
            D_sb = consts.tile([128, NJH, 1], F32, tag="D")
            nc.sync.dma_start(out=D_sb, in_=D_col_d[:].rearrange("(j p) o -> p j o", p=128))
            ones_r = consts.tile([128, 1], F32R, tag="ones")
            nc.sync.dma_start(out=ones_r, in_=ones_d[:].bitcast(F32R))
            ident = consts.tile([128, 128], FP16, tag="ident")
            nc.sync.dma_start(out=ident, in_=ident_d[:])
            xprojT = consts.tile([128, NJ, 96], F32R, tag="xprojT")
            nc.sync.dma_start(out=xprojT, in_=xprojT_d[:].rearrange("(j p) c -> p j c", p=128).bitcast(F32R))
            dtwT = consts.tile([DT_RANK, NJH, 128], F32R, tag="dtwT")
            nc.sync.dma_start(out=dtwT, in_=dtwT_d[:].rearrange("r (j p) -> r j p", p=128).bitcast(F32R))

            # ---- persistent activations ----
            u_keep = persist.tile([128, NJH, L], F32R, tag="u_keep")       # 4MB
            z_silu = persist.tile([128, NJH, L], FP16, tag="z_silu")       # 2MB
            yg = u_keep  # gated output overwrites u in place (u dead after g1)
            xdbl = persist.tile([96, L], F32R, tag="xdbl")
            bc_dram = dramp.tile([32, L], FP16, tag="bc_dram")

            # =========== Stage A: LayerNorm (layout [m, t]), in-place ===========
            with tc.tile_pool(name="stageXN", bufs=1) as pXN, \
                 tc.tile_pool(name="stageA", bufs=1) as pA, \
                 tc.tile_pool(name="stageAs", bufs=2) as pAs, \
                 tc.tile_pool(name="psLN", bufs=1, space="PSUM") as psLN:
                xsb = pXN.tile([128, NK, L], F32R, tag="xsb")
                nc.sync.dma_start(out=xsb, in_=xT_d[:].rearrange("(k p) t -> p k t", p=128).bitcast(F32R))
                xn = xsb

                ps_mu = [psLN.tile([1, TC], F32, tag=f"psmu{t}", name=f"ps_mu{t}") for t in range(2)]
                ps_s2 = [psLN.tile([1, TC], F32, tag=f"pss2{t}", name=f"ps_s2{t}") for t in range(2)]
                for t in range(2):
                    for k in range(NK):
                        nc.tensor.matmul(ps_mu[t][:], ones_r[:], xsb[:, k, t * TC:(t + 1) * TC],
                                         start=(k == 0), stop=(k == NK - 1))
                for k in range(NK):
                    sq = pAs.tile([128, L], F32R, tag="sq")
                    nc.scalar.activation(sq[:], xsb[:, k, :].bitcast(F32), ACTF.Square)
                    for t in range(2):
                        nc.tensor.matmul(ps_s2[t][:], ones_r[:], sq[:, t * TC:(t + 1) * TC],
                                         start=(k == 0), stop=(k == NK - 1))
                stat_mu = pA.tile([1, L], F32, tag="stat_mu")
                stat_ms = pA.tile([1, L], F32, tag="stat_ms")
                for t in range(2):
                    nc.scalar.activation(stat_mu[:, t * TC:(t + 1) * TC], ps_mu[t][:], ACTF.Copy,
                                         scale=1.0 / D_MODEL)
                    nc.scalar.activation(stat_ms[:, t * TC:(t + 1) * TC], ps_s2[t][:], ACTF.Copy,
                                         scale=1.0 / D_MODEL)
                mu2 = pA.tile([1, L], F32, tag="mu2")
                nc.scalar.activation(mu2[:], stat_mu[:], ACTF.Square)
                var = pA.tile([1, L], F32, tag="var")
                nc.vector.tensor_tensor(var[:], stat_ms[:], mu2[:], ALU.subtract)
                eps = pA.tile([1, 1], F32, tag="eps")
                nc.vector.memset(eps[:], 1e-5)
                lv = pA.tile([1, L], F32, tag="lv")
                nc.scalar.activation(lv[:], var[:], ACTF.Ln, bias=eps[:])
                rs = pA.tile([1, L], F32, tag="rs")
                nc.scalar.activation(rs[:], lv[:], ACTF.Exp, scale=-0.5)
                nmurs = pA.tile([1, L], F32, tag="nmurs")
                nc.vector.tensor_tensor(nmurs[:], stat_mu[:], rs[:], ALU.mult)
                nc.scalar.activation(nmurs[:], nmurs[:], ACTF.Copy, scale=-1.0)
                rs_b = pA.tile([128, L], F32, tag="rs_b")
                nc.gpsimd.partition_broadcast(rs_b[:], rs[:])
                nm_b = pA.tile([128, L], F32, tag="nm_b")
                nc.gpsimd.partition_broadcast(nm_b[:], nmurs[:])
                for k in range(NK):
                    c1 = pAs.tile([128, L], F32, tag="c1")
                    nc.vector.tensor_tensor(c1[:], xsb[:, k, :].bitcast(F32), rs_b[:], ALU.mult)
                    nc.vector.tensor_tensor(c1[:], c1[:], nm_b[:], ALU.add)
                    nc.scalar.activation(xn[:, k, :], c1[:], ACTF.Identity,
                                         bias=beta[:, k, :], scale=gamma[:, k, :])

                # ====== Stage B: in_proj + conv + silu + xproj accum + z ======
                with tc.tile_pool(name="stageB", bufs=3) as pB, \
                     tc.tile_pool(name="stageB2", bufs=2) as pB2, \
                     tc.tile_pool(name="psPX", bufs=2, space="PSUM") as psPX:
                    px = [psPX.tile([96, TC], F32, tag=f"px{t}", name=f"px{t}") for t in range(2)]
                    for j in range(NJ):
                        wj = pB.tile([128, NK, 128], F32R, tag="wj")
                        nc.sync.dma_start(
                            out=wj, in_=wuT_d[:, j * 128:(j + 1) * 128]
                            .rearrange("(k p) d -> p k d", p=128).bitcast(F32R))
                        upre = pB2.tile([128, D_CONV - 1 + L], F32R, tag="upre")
                        nc.vector.memset(upre[:, 0:D_CONV - 1].bitcast(F32), 0.0)
                        for t in range(2):
                            pu = psA.tile([128, TC], F32, tag="pu")
                            for k in range(NK):
                                nc.tensor.matmul(pu[:], wj[:, k, :], xn[:, k, t * TC:(t + 1) * TC],
                                                 start=(k == 0), stop=(k == NK - 1))
                            nc.scalar.activation(upre[:, 3 + t * TC: 3 + (t + 1) * TC], pu[:], ACTF.Copy)
                        convd = pB.tile([128, D_CONV, 128], F32R, tag="convd")
                        nc.sync.dma_start(
                            out=convd,
                            in_=convdiag_d[j].rearrange("q r c -> r q c").bitcast(F32R))
                        if j < NJH:
                            u_j = u_keep[:, j, :]
                        else:
                            u_jt = pB.tile([128, L], F32R, tag="u_tmp", bufs=2)
                            u_j = u_jt[:]
                        for t in range(2):
                            pc = psA.tile([128, TC], F32, tag="pu")
                            for q in range(D_CONV):
                                nc.tensor.matmul(pc[:], convd[:, q, :],
                                                 upre[:, q + t * TC: q + t * TC + TC],
                                                 start=(q == 0), stop=(q == D_CONV - 1))
                            nc.scalar.activation(u_j[:, t * TC:(t + 1) * TC], pc[:], ACTF.Silu,
                                                 bias=convb[:, j, :])
                        for t in range(2):
                            nc.tensor.matmul(px[t][:], xprojT[:, j, :], u_j[:, t * TC:(t + 1) * TC],
                                             start=(j == 0), stop=(j == NJ - 1))
                    for j in range(NJH):
                        wzj = pB.tile([128, NK, 128], F32R, tag="wj")
                        nc.sync.dma_start(
                            out=wzj, in_=wzT_d[:, j * 128:(j + 1) * 128]
                            .rearrange("(k p) d -> p k d", p=128).bitcast(F32R))
                        for t in range(2):
                            pz = psA.tile([128, TC], F32, tag="pu")
                            for k in range(NK):
                                nc.tensor.matmul(pz[:], wzj[:, k, :], xn[:, k, t * TC:(t + 1) * TC],
                                                 start=(k == 0), stop=(k == NK - 1))
                            if not silu_compat:
                              nc.scalar.activation(z_silu[:, j, t * TC:(t + 1) * TC], pz[:], ACTF.Silu)
                          else:
                              sgv = pB.tile([128, TC], F32, tag="sgv", bufs=2)
                              xv = pB.tile([128, TC], F32, tag="xv", bufs=2)
                              nc.scalar.activation(sgv[:], pz[:], ACTF.Sigmoid)
                              nc.scalar.activation(xv[:], pz[:], ACTF.Identity)
                              nc.vector.tensor_tensor(z_silu[:, j, t * TC:(t + 1) * TC], xv[:], sgv[:], ALU.mult)

                    # Stage C: x_dbl out of psum; B/C rows to DRAM for broadcast
                    bcm = pB.tile([96, L], FP16, tag="bcm", bufs=1)
                    for t in range(2):
                        nc.scalar.activation(xdbl[:, t * TC:(t + 1) * TC], px[t][:], ACTF.Copy)
                        nc.scalar.activation(bcm[64:96, t * TC:(t + 1) * TC], px[t][64:96, :], ACTF.Copy)
                    nc.sync.dma_start(out=bc_dram[:], in_=bcm[64:96, :])

            # ====== Stage D/E: dt + scan ======
            with tc.tile_pool(name="bcres", bufs=1) as pBC, \
                 tc.tile_pool(name="scan", bufs=3) as pS, \
                 tc.tile_pool(name="scan4", bufs=4) as pS4, \
                 tc.tile_pool(name="psy", bufs=2, space="PSUM") as psY:
                Bm_r = pBC.tile([128, D_STATE, L], FP16, tag="Bm_r")
                Cm_r = pBC.tile([128, D_STATE, L], FP16, tag="Cm_r")
                for n in range(D_STATE):
                    nc.sync.dma_start(out=Bm_r[:, n, :], in_=bc_dram[n:n + 1, :].to_broadcast([128, L]))
                    nc.sync.dma_start(out=Cm_r[:, n, :], in_=bc_dram[16 + n:17 + n, :].to_broadcast([128, L]))
                for j in range(NJH):
                    dt_j = pS.tile([128, L], F32, tag="dt", bufs=2)
                    for t in range(2):
                        pdt = psA.tile([128, TC], F32, tag="pu")
                        nc.tensor.matmul(pdt[:], dtwT[:, j, :], xdbl[0:DT_RANK, t * TC:(t + 1) * TC],
                                         start=True, stop=True)
                        # softplus(x) = ln(1 + exp(x)); no Softplus LUT in this toolchain
                        edt = pS.tile([128, TC], F32, tag="edt", bufs=2)
                        nc.scalar.activation(edt[:], pdt[:], ACTF.Exp, bias=dtb[:, j, :])
                        nc.scalar.activation(dt_j[:, t * TC:(t + 1) * TC], edt[:], ACTF.Ln, bias=1.0)
                    dtu_j = pS.tile([128, L], FP16, tag="dtu", bufs=2)
                    nc.vector.tensor_tensor(dtu_j[:], dt_j[:], u_keep[:, j, :].bitcast(F32), ALU.mult)

                    py = psY.tile([128, L], F32, tag="py")
                    for n in range(D_STATE):
                        dA = pS.tile([128, L], FP16, tag="dA")
                        nc.scalar.activation(dA[:], dt_j[:], ACTF.Exp, scale=A_sb[:, j, n:n + 1])
                        dBu = pS.tile([128, L], FP16, tag="dBu")
                        nc.vector.tensor_tensor(dBu[:], dtu_j[:], Bm_r[:, n, :], ALU.mult)
                        h = pS.tile([128, L], FP16, tag="h")
                        nc.vector.tensor_tensor_scan(h[:], dA[:], dBu[:], 0.0, ALU.mult, ALU.add)
                        hC = pS4.tile([128, L], FP16, tag="hC")
                        if n < 12:
                            nc.gpsimd.tensor_tensor(hC[:], h[:], Cm_r[:, n, :], ALU.mult)
                        else:
                            nc.vector.tensor_tensor(hC[:], h[:], Cm_r[:, n, :], ALU.mult)
                        for t in range(2):
                            nc.tensor.matmul(py[:, t * TC:(t + 1) * TC], ident[:],
                                             hC[:, t * TC:(t + 1) * TC],
                                             start=(n == 0), stop=(n == D_STATE - 1))
                    g1 = pS.tile([128, L], F32, tag="g1", bufs=2)
                    nc.vector.scalar_tensor_tensor(g1[:], u_keep[:, j, :].bitcast(F32), D_sb[:, j, :],
                                                   py[:], ALU.mult, ALU.add)
                    nc.vector.tensor_tensor(yg[:, j, :], g1[:], z_silu[:, j, :], ALU.mult)

            # ====== Stage F: out_proj ======
            with tc.tile_pool(name="stageF", bufs=1) as pF, \
                 tc.tile_pool(name="stageFo", bufs=3) as pFo:
                owT = pF.tile([128, NJH, D_MODEL], F32R, tag="owT")
                nc.sync.dma_start(out=owT, in_=owT_d[:].rearrange("(j p) e -> p j e", p=128).bitcast(F32R))
                for t in range(NK):  # 8 t-chunks of 128
                    ost = pFo.tile([128, D_MODEL], F32, tag="ost")
                    for e in range(2):
                        po = psA.tile([128, TC], F32, tag="pu")
                        for j in range(NJH):
                            nc.tensor.matmul(po[:], yg[:, j, t * 128:(t + 1) * 128],
                                             owT[:, j, e * TC:(e + 1) * TC],
                                             start=(j == 0), stop=(j == NJH - 1))
                        nc.scalar.activation(ost[:, e * TC:(e + 1) * TC], po[:], ACTF.Copy)
                    nc.sync.dma_start(out=outp_d[t * 128:(t + 1) * 128, :], in_=ost[:])

    nc.compile()
    return nc


# ---------------------------------------------------------------------------
# host side
# ---------------------------------------------------------------------------

def _prep_core_inputs(inputs, branch, b, h):
    p = "f_" if branch == 0 else "b_"
    x = np.asarray(inputs["x"], dtype=np.float32)
    xb = x[b]
    if branch == 1:
        xb = xb[::-1]
    in_w = np.asarray(inputs[p + "in_w"], np.float32)
    conv_w = np.asarray(inputs[p + "conv_w"], np.float32)
    conv_b = np.asarray(inputs[p + "conv_b"], np.float32)
    xproj_w = np.asarray(inputs[p + "xproj_w"], np.float32)
    dt_w = np.asarray(inputs[p + "dt_w"], np.float32)
    dt_b = np.asarray(inputs[p + "dt_b"], np.float32)
    A_log = np.asarray(inputs[p + "A_log"], np.float32)
    D = np.asarray(inputs[p + "D"], np.float32)
    out_w = np.asarray(inputs[p + "out_w"], np.float32)

    sl = slice(h * HALF, (h + 1) * HALF)
    # permute d_inner so our half comes first
    perm = np.concatenate([np.arange(h * HALF, (h + 1) * HALF),
                           np.arange((1 - h) * HALF, (2 - h) * HALF)])
    wu = in_w[:D_INNER][perm]                  # [D_INNER, D_MODEL]
    cw = conv_w[:, 0, :][perm]                 # [D_INNER, 4]
    cb = conv_b[perm]
    xp = xproj_w[:, perm]                      # [96, D_INNER]

    convdiag = np.zeros((NJ, D_CONV, 128, 128), np.float32)
    idx = np.arange(128)
    for j in range(NJ):
        for q in range(D_CONV):
            convdiag[j, q, idx, idx] = cw[j * 128:(j + 1) * 128, q]

    return dict(
        xT=np.ascontiguousarray(xb.T),
        gamma_col=np.ascontiguousarray(np.asarray(inputs["gamma"], np.float32)[:, None]),
        beta_col=np.ascontiguousarray(np.asarray(inputs["beta"], np.float32)[:, None]),
        wuT=np.ascontiguousarray(wu.T),
        wzT=np.ascontiguousarray(in_w[D_INNER + h * HALF: D_INNER + (h + 1) * HALF].T),
        convdiag=convdiag,
        convb_col=np.ascontiguousarray(cb[:, None]),
        xprojT=np.ascontiguousarray(xp.T),
        dtwT=np.ascontiguousarray(dt_w[sl].T),
        dtb_col=np.ascontiguousarray(dt_b[sl][:, None]),
        A_half=np.ascontiguousarray(-np.exp(A_log[sl])),
        D_col=np.ascontiguousarray(D[sl][:, None]),
        owT=np.ascontiguousarray(out_w[:, sl].T),
        ones_col=np.ones((128, 1), np.float32),
        ident_bf=np.eye(128, dtype=np.float32),  # cast to bf16 in make_in_maps
    )


_RUNNER = None


class _Runner:
    def __init__(self):
        self.nc = build_program()
        self._jit = None
        self._meta = None

    def _build_jit(self):
        import jax
        from jax.sharding import Mesh, PartitionSpec
        from jax.experimental.shard_map import shard_map
        from concourse import mybir as _mybir
        from concourse.bass2jax import (_bass_exec_p, install_neuronx_cc_hook,
                                        partition_id_tensor)

        install_neuronx_cc_hook()
        nc = self.nc
        partition_name = nc.partition_id_tensor.name if nc.partition_id_tensor else None
        dbg_name = nc.dbg_addr.name if nc.dbg_addr is not None else None
        in_names, out_names, out_avals, zero_outs = [], [], [], []
        for alloc in nc.m.functions[0].allocations:
            if not isinstance(alloc, _mybir.MemoryLocationSet):
                continue
            name = alloc.memorylocations[0].name
            if alloc.kind == "ExternalInput":
                if name != partition_name:
                    in_names.append(name)
            elif alloc.kind == "ExternalOutput":
                out_names.append(name)
                shape = tuple(alloc.tensor_shape)
                dtype = _mybir.dt.np(alloc.dtype)
                out_avals.append(jax.core.ShapedArray(shape, dtype))
                zero_outs.append(np.zeros(shape, dtype))
        n_params = len(in_names)
        all_in_names = list(in_names) + list(out_names)
        if partition_name is not None:
            all_in_names.append(partition_name)

        def _body(*args):
            operands = list(args)
            if partition_name is not None:
                operands.append(partition_id_tensor())
            outs = _bass_exec_p.bind(
                *operands,
                out_avals=tuple(out_avals),
                in_names=tuple(all_in_names),
                out_names=tuple(out_names),
                lowering_input_output_aliases=(),
                sim_require_finite=True,
                sim_require_nnan=True,
                nc=nc,
            )
            return tuple(outs)

        devices = jax.devices()[:N_CORES]
        mesh = Mesh(np.asarray(devices), ("core",))
        self._mesh = mesh
        in_specs = (PartitionSpec("core"),) * (n_params + len(out_names))
        out_specs = (PartitionSpec("core"),) * len(out_names)
        self._jit = jax.jit(
            shard_map(_body, mesh=mesh, in_specs=in_specs, out_specs=out_specs,
                      check_rep=False),
            keep_unused=True)
        self._meta = (in_names, out_names, out_avals, zero_outs, dbg_name)

    def _concat_inputs(self, in_maps):
        in_names, out_names, out_avals, zero_outs, dbg_name = self._meta
        maps = in_maps
        if dbg_name is not None:
            maps = [{**m, dbg_name: np.zeros((1, 2), np.uint32)} for m in in_maps]
        concat_in = [np.concatenate([np.asarray(maps[c][nm]) for c in range(N_CORES)], axis=0)
                     for nm in in_names]
        concat_zero = [np.zeros((N_CORES * z.shape[0], *z.shape[1:]), z.dtype) for z in zero_outs]
        return concat_in, concat_zero

    def put_device_inputs(self, in_maps):
        """device_put the concatenated inputs with the mesh sharding so that
        repeated executions don't re-ship inputs over the wire."""
        import jax
        from jax.sharding import NamedSharding, PartitionSpec
        if self._jit is None:
            self._build_jit()
        concat_in, concat_zero = self._concat_inputs(in_maps)
        sh = NamedSharding(self._mesh, PartitionSpec("core"))
        dev_in = [jax.device_put(a, sh) for a in concat_in]
        dev_zero = [jax.device_put(a, sh) for a in concat_zero]
        for a in dev_in + dev_zero:
            a.block_until_ready()
        return dev_in, dev_zero

    def execute(self, dev_in, concat_zero):
        """Run with device-resident inputs; returns raw jax output arrays."""
        out_arrs = self._jit(*dev_in, *concat_zero)
        for a in out_arrs:
            a.block_until_ready()
        return out_arrs

    def split_outputs(self, out_arrs):
        in_names, out_names, out_avals, zero_outs, dbg_name = self._meta
        res = []
        for c in range(N_CORES):
            res.append({nm: np.asarray(out_arrs[i]).reshape(N_CORES, *out_avals[i].shape)[c]
                        for i, nm in enumerate(out_names)})
        return res

    def run(self, in_maps):
        if self._jit is None:
            self._build_jit()
        concat_in, concat_zero = self._concat_inputs(in_maps)
        return self.split_outputs(self._jit(*concat_in, *concat_zero))



def get_runner():
    global _RUNNER
    if _RUNNER is None:
        _RUNNER = _Runner()
    return _RUNNER


def make_in_maps(inputs):
    import ml_dtypes
    in_maps = []
    for branch in range(2):
        for b in range(BATCH):
            for h in range(2):
                ci = _prep_core_inputs(inputs, branch, b, h)
                ci["ident_bf"] = ci["ident_bf"].astype(np.float16)
                in_maps.append(ci)
    return in_maps


def assemble(inputs, results):
    x = np.asarray(inputs["x"], np.float32)
    out = np.array(x, copy=True)
    c = 0
    for branch in range(2):
        for b in range(BATCH):
            for h in range(2):
                p = results[c]["outp"]
                if branch == 1:
                    p = p[::-1]
                out[b] += p
                c += 1
    return out


def kernel(**inputs):
    runner = get_runner()
    in_maps = make_in_maps(inputs)
    results = runner.run(in_maps)
    return assemble(inputs, results)
